# revision 74
# baseline (speedup 1.0000x reference)
"""AdaLN attention block (DiT-style) on 8 TRN2 NeuronCores.

Sharding: 8 cores = 4 batches x 2 token-halves, no collectives. Core c handles
batch c//2 and query-token half c%2: layernorm1 and k/v are computed over the
full (permuted) sequence, everything else only for the own 512 query rows.

Cost-model reality (measured): matmul time = out_free_size x 1 cycle
regardless of dtype (DoubleRow gets no 0.5 discount; f32 pays 4x), so the
only PE lever is matmul COUNT. fp8 DoubleRow still halves the count for
K-contractions (K=256 per matmul):
- q/k/v projections: fp8 h (x1) against fp8 weights (x16), 1-term DR -> half
  the matmuls of bf16. q/k evicted to bf16 x16 tiles; sim runs bf16 (K=64,
  same cost as fp8 here, better numerics).
- attn@v contracts key tiles as DR pairs: exp is written by ACT directly as
  fp8 into key-pair slots (exp(sim*s - 6) fits e4m3's 240 max), v fp8 x16
  with an exact fp8 ones-column feeding the softmax denominator; 1/den is
  partition-broadcast on GPSIMD (no f32 matmul, no PSUM copies).
- LN stats contract via fp8 DoubleRow pairs of bf16->fp8 x/x^2 copies.
- MLP and Wo stay bf16 (3-term fp8 would cost MORE matmuls than bf16).
- Bias algebra folded host-side: bk dropped (cancels in softmax), bv folded
  into bo_eff = bv@Wo + bo.
- Weights are re-laid-out host-side to [128, ...] partition-major so each
  stream is a few large contiguous DMAs.
"""

import numpy as np
from contextlib import ExitStack

import concourse.bass as bass
import concourse.bacc as bacc
import concourse.mybir as mybir
from concourse import tile
from concourse.tile import add_dep_helper
from concourse.bass_utils import run_bass_kernel_spmd

P = 128
D = 1024
N = 1024
NQ = 512
H = 16
DH = 64
MLPD = 4096
EPS = 1e-6
NCORES = 8
SHIFT = 6.0                      # softmax shift so exp fits e4m3 (max 240)
SSCALE = (DH ** -0.5) / 256.0    # q16.k16 psum -> sim

F32 = mybir.dt.float32
BF16 = mybir.dt.bfloat16
FP8 = mybir.dt.float8e4
AF = mybir.ActivationFunctionType
ALU = mybir.AluOpType
DR = mybir.MatmulPerfMode.DoubleRow

KT = D // P            # 8 contraction tiles over D
NJ = KT // 2           # 4 k-tile pairs
MT = MLPD // P         # 32 tiles over MLP dim
GRP = MLPD // NQ       # 8 MLP column groups


def build():
    nc = bacc.Bacc("TRN2", target_bir_lowering=False, debug=False,
                   num_devices=NCORES)

    xbT = nc.dram_tensor("xbT", [D, N], BF16, kind="ExternalInput")
    xoT = nc.dram_tensor("xoT", [D, NQ], F32, kind="ExternalInput")
    crow = nc.dram_tensor("crow", [1, D], F32, kind="ExternalInput")
    # fp8 projection weights (x16) in [128, ...] partition-major DR layout
    Wqd = nc.dram_tensor("Wqd", [P, NJ * 2 * D], FP8, kind="ExternalInput")
    Wkvd = nc.dram_tensor("Wkvd", [P, NJ * 4 * D], FP8,
                          kind="ExternalInput")
    Wo = nc.dram_tensor("Wo", [D, D], BF16, kind="ExternalInput")
    # W1 bf16 [128, (grp, k, c)]; W2 bf16 [128, (half, mk, c)]
    W1r = nc.dram_tensor("W1r", [P, D * MLPD // P], BF16,
                         kind="ExternalInput")
    W2r = nc.dram_tensor("W2r", [P, MLPD * D // P], BF16,
                         kind="ExternalInput")
    Wada = nc.dram_tensor("Wada", [D, 6 * D], BF16, kind="ExternalInput")
    bada_r = nc.dram_tensor("bada_r", [1, 6 * D], BF16, kind="ExternalInput")
    bq16_c = nc.dram_tensor("bq16_c", [P, KT], F32, kind="ExternalInput")
    bo16_c = nc.dram_tensor("bo16_c", [P, KT], F32, kind="ExternalInput")
    b1_c = nc.dram_tensor("b1_c", [P, MT], F32, kind="ExternalInput")
    b2_c = nc.dram_tensor("b2_c", [P, KT], F32, kind="ExternalInput")
    yT = nc.dram_tensor("yT", [D, NQ], F32, kind="ExternalOutput")

    with tile.TileContext(nc) as tc, ExitStack() as root:
        const = root.enter_context(tc.tile_pool(name="const", bufs=1))
        rootrows = root.enter_context(tc.tile_pool(name="rootrows", bufs=1))

        # c row first: the silu/csT chain gates the mod computation
        c_sb = rootrows.tile([1, D], F32, name='c_sb')
        nc.sync.dma_start(c_sb[:], crow[:])

        ones_col_b = const.tile([P, 1], BF16, name='ones_col_b')
        nc.vector.memset(ones_col_b[:], 1.0)
        ones_row = const.tile([1, 1], BF16, name='ones_row')
        nc.vector.memset(ones_row[:], 1.0)
        ones_rowq = const.tile([1, NQ], BF16, name='ones_rowq')
        nc.vector.memset(ones_rowq[:], 1.0)
        eps_t = const.tile([1, 1], F32, name='eps_t')
        nc.vector.memset(eps_t[:], EPS)
        nshift_t = const.tile([P, 1], F32, name='nshift_t')
        nc.vector.memset(nshift_t[:], -SHIFT)

        bq16T = const.tile([P, KT], F32, name='bq16T')
        bo16T = const.tile([P, KT], F32, name='bo16T')
        b1T = const.tile([P, MT], F32, name='b1T')
        b2T = const.tile([P, KT], F32, name='b2T')
        nc.gpsimd.dma_start(bq16T[:], bq16_c[:])
        nc.gpsimd.dma_start(bo16T[:], bo16_c[:])
        nc.gpsimd.dma_start(b1T[:], b1_c[:])
        nc.gpsimd.dma_start(b2T[:], b2_c[:])

        csT = const.tile([P, KT], BF16, name='csT')
        gmsaT = const.tile([P, KT], F32, name='gmsaT')   # g_msa / 16
        gmlpT = const.tile([P, KT], F32, name='gmlpT')   # g_mlp

        # left-side persistent pools; pop order: hp (P3), op (P4), x (P4)
        x_cm = tc.tile_pool(name="xp", bufs=1, side='left')
        xp_ = x_cm.__enter__()
        xt = [xp_.tile([P, N], BF16, tag=f"x{k}", name=f"x{k}")
              for k in range(KT)]
        xo = [xp_.tile([P, NQ], F32, tag=f"xo{k}", name=f"xo{k}")
              for k in range(KT)]
        for k in range(KT):
            nc.sync.dma_start(xt[k][:], xbT[k * P:(k + 1) * P, :])

        op_cm = tc.tile_pool(name="op", bufs=1, side='left')
        op_ = op_cm.__enter__()
        outT = [op_.tile([P, NQ], BF16, tag=f"o{k}", name=f"o{k}")
                for k in range(KT)]

        hp_cm = tc.tile_pool(name="hp", bufs=1, side='left')
        hp_ = hp_cm.__enter__()
        # h as fp8 k-tile pairs: tile j = k-tiles (2j: cols 0:N, 2j+1: N:2N)
        hpr = [hp_.tile([P, 2 * N], FP8, tag=f"h{j}", name=f"h{j}")
               for j in range(NJ)]

        mod_row = rootrows.tile([1, 6 * D], BF16, name='mod_row')

        # right-side pools; wop (root) at the bottom, the rest popped
        # after P2/P3 in reverse creation order
        wop = root.enter_context(tc.tile_pool(name="wop", bufs=1,
                                              side='right'))
        wkv_cm = tc.tile_pool(name="wkvp", bufs=1, side='right')
        wkvp = wkv_cm.__enter__()
        wkv_t = wkvp.tile([P, NJ * 4 * D], FP8, name='wkv_t')
        HKV = NJ * 2 * D
        wkv4 = wkv_t.rearrange("p (j t c) -> p j t c", j=NJ, t=2)

        qkv_cm = tc.tile_pool(name="qkvp", bufs=1, side='right')
        qkvp = qkv_cm.__enter__()
        bada_sb = qkvp.tile([1, 6 * D], BF16, name='bada_sb')
        nc.gpsimd.dma_start(bada_sb[:], bada_r[:])

        def h3(j):
            return hpr[j].rearrange("p (t n) -> p t n", t=2)

        last_dma = {0: None, 1: None}

        def chained_dma(dst, src_ap, e=None):
            if e is None:
                e = chained_dma.flip
                chained_dma.flip ^= 1
            eng = nc.sync if e == 0 else nc.gpsimd
            di = eng.dma_start(dst, src_ap)
            if last_dma[e] is not None:
                add_dep_helper(di.ins, last_dma[e].ins, sync=True,
                               reason="prefetch ordered after critical DMAs")
            last_dma[e] = di
            return di
        chained_dma.flip = 0

        # ---------------- phase 0: mod groups 0-3 & ln1 stats ---------------
        p0_cm = tc.tile_pool(name="p0rows", bufs=1)
        rows = p0_cm.__enter__()
        sq_cm = tc.tile_pool(name="p0sq", bufs=2)
        sqpool = sq_cm.__enter__()

        with ExitStack() as sec:
            wpool = sec.enter_context(tc.tile_pool(name="p0w", bufs=4))
            pscol = sec.enter_context(
                tc.tile_pool(name="pscol", bufs=1, space="PSUM"))
            psmod = sec.enter_context(
                tc.tile_pool(name="psmod", bufs=1, space="PSUM"))
            psstat = sec.enter_context(
                tc.tile_pool(name="psstat", bufs=1, space="PSUM"))

            # silu(c) and its column layout
            cs_row = rows.tile([1, D], BF16, name='cs_row')
            nc.scalar.activation(cs_row[:], c_sb[:], AF.Silu)
            psc = pscol.tile([P, KT], F32, tag="colps", name='colps')
            for j in range(KT):
                nc.tensor.matmul(psc[:, j:j + 1],
                                 lhsT=cs_row[0:1, j * P:(j + 1) * P],
                                 rhs=ones_row[0:1, 0:1],
                                 start=True, stop=True)
            nc.vector.tensor_copy(csT[:], psc[:])

            # mod groups 0-3 = sh_msa | sc_msa
            mps = [psmod.tile([1, NQ], F32, tag=f"modps{g}",
                              name=f"modps{g}") for g in range(4)]
            for k in range(KT):
                wch4 = wpool.tile([P, 4 * NQ], BF16, tag="wada",
                                  name='wada')
                di = nc.gpsimd.dma_start(wch4[:],
                                         Wada[k * P:(k + 1) * P, 0:4 * NQ])
                last_dma[1] = di
                for g in range(4):
                    nc.tensor.matmul(
                        mps[g][:], lhsT=csT[:, k:k + 1],
                        rhs=wch4[:, g * NQ:(g + 1) * NQ],
                        start=(k == 0), stop=(k == KT - 1))
            for g in range(4):
                nc.vector.tensor_add(
                    mod_row[0:1, g * NQ:(g + 1) * NQ], mps[g][:],
                    bada_sb[0:1, g * NQ:(g + 1) * NQ])

            # ln1 stats: sum and sumsq over d via bf16 x copies (ACT)
            mu_row = rows.tile([1, N], F32, name='mu_row')
            ex2_row = rows.tile([1, N], F32, name='ex2_row')
            for ch in range(2):
                sl = slice(ch * NQ, (ch + 1) * NQ)
                ss = psstat.tile([1, NQ], F32, tag="st_s", name='st_s')
                sq_ps = psstat.tile([1, NQ], F32, tag="st_q", name='st_q')
                for k in range(KT):
                    sq = sqpool.tile([P, NQ], BF16, tag="xsq", name='xsq')
                    nc.scalar.activation(sq[:], xt[k][:, sl], AF.Square)
                    nc.tensor.matmul(ss[:], lhsT=ones_col_b[:],
                                     rhs=xt[k][:, sl],
                                     start=(k == 0), stop=(k == KT - 1))
                    nc.tensor.matmul(sq_ps[:], lhsT=ones_col_b[:],
                                     rhs=sq[:],
                                     start=(k == 0), stop=(k == KT - 1))
                nc.vector.tensor_scalar_mul(mu_row[0:1, sl], ss[:], 1.0 / D)
                nc.vector.tensor_scalar_mul(ex2_row[0:1, sl], sq_ps[:],
                                            1.0 / D)

            scr_row = rows.tile([1, N], F32, name='scr_row')
            a_row = rows.tile([1, N], BF16, name='a_row')
            b_row = rows.tile([1, N], BF16, name='b_row')
            S1_row = rows.tile([1, D], BF16, name='S1_row')
            nc.vector.tensor_scalar_add(S1_row[:], mod_row[0:1, D:2 * D],
                                        1.0)
            for ch in range(2):
                sl = slice(ch * NQ, (ch + 1) * NQ)
                nc.vector.tensor_mul(scr_row[0:1, sl], mu_row[0:1, sl],
                                     mu_row[0:1, sl])
                nc.vector.tensor_sub(ex2_row[0:1, sl], ex2_row[0:1, sl],
                                     scr_row[0:1, sl])
                nc.scalar.activation(scr_row[0:1, sl], ex2_row[0:1, sl],
                                     AF.Sqrt, bias=eps_t[:])
                nc.vector.reciprocal_approx_fast(ex2_row[0:1, sl],
                                                 scr_row[0:1, sl])
                nc.vector.tensor_copy(a_row[0:1, sl], ex2_row[0:1, sl])
                nc.vector.scalar_tensor_tensor(b_row[0:1, sl],
                                               mu_row[0:1, sl], -1.0,
                                               ex2_row[0:1, sl],
                                               ALU.mult, ALU.mult)

        # q weights on top of the right stack (popped after P3)
        wq_cm = tc.tile_pool(name="wqp", bufs=1, side='right')
        wqp = wq_cm.__enter__()
        wq_t = wqp.tile([P, NJ * 2 * D], FP8, name='wq_t')
        nc.scalar.dma_start(wq_t[:], Wqd[:])
        nc.scalar.dma_start(wkv_t[:, 0:HKV], Wkvd[:, 0:HKV])
        nc.scalar.dma_start(wkv_t[:, HKV:2 * HKV], Wkvd[:, HKV:2 * HKV])
        wq4 = wq_t.rearrange("p (j t c) -> p j t c", j=NJ, t=2)

        # ------------- phase 1: h = fp8(x*A + B) ----------------
        with ExitStack() as sec:
            psab = sec.enter_context(
                tc.tile_pool(name="psab", bufs=2, space="PSUM"))
            for k in range(KT):
                for ch in range(2):
                    sl = slice(ch * NQ, (ch + 1) * NQ)
                    pa = psab.tile([P, NQ], F32, tag="pA", name='pA')
                    pb = psab.tile([P, NQ], F32, tag="pB", name='pB')
                    nc.tensor.matmul(
                        pa[:], lhsT=S1_row[0:1, k * P:(k + 1) * P],
                        rhs=a_row[0:1, sl], start=True, stop=True)
                    nc.tensor.matmul(
                        pb[:], lhsT=S1_row[0:1, k * P:(k + 1) * P],
                        rhs=b_row[0:1, sl], start=True, stop=False)
                    nc.tensor.matmul(
                        pb[:], lhsT=mod_row[0:1, k * P:(k + 1) * P],
                        rhs=ones_rowq[:], start=False, stop=True)
                    htmp = sqpool.tile([P, NQ], F32, tag="htmp",
                                       name='htmp')
                    nc.vector.tensor_mul(htmp[:], xt[k][:, sl], pa[:])
                    base = (k % 2) * N
                    nc.vector.tensor_add(
                        hpr[k // 2][:, base + ch * NQ:base + (ch + 1) * NQ],
                        htmp[:], pb[:])

        sq_cm.__exit__(None, None, None)
        p0_cm.__exit__(None, None, None)

        # ---------------- phase 2/3 shared: q/k/v emission ----------------
        # q/k bf16 x16 feature-major tiles (2 heads per tile); v fp8 x16
        # transposed (keys on partitions) as key-pair tiles with a fp8
        # ones column per head feeding the softmax denominator
        qTt = [qkvp.tile([P, NQ], BF16, tag=f"q{i}", name=f"q{i}")
               for i in range(KT)]
        kTt = [qkvp.tile([P, N], BF16, tag=f"k{i}", name=f"k{i}")
               for i in range(KT)]
        vpr = [qkvp.tile([P, 2 * H * (DH + 1)], FP8, tag=f"v{j}",
                         name=f"v{j}") for j in range(NJ)]

        prj_cm = tc.tile_pool(name="prjps", bufs=2, space="PSUM",
                              side='right')
        prjps = prj_cm.__enter__()

        def v4(j):
            return vpr[j].rearrange("p (t h w) -> p t h w", t=2, w=DH + 1)

        def emit_q(tau):
            p = prjps.tile([P, NQ], F32, tag="prj", name='prj')
            for j in range(NJ):
                nc.tensor.matmul(
                    p[:], lhsT=wq4[:, j, :, tau * P:(tau + 1) * P],
                    rhs=h3(j)[:, :, 0:NQ],
                    start=(j == 0), stop=(j == NJ - 1), perf_mode=DR)
            nc.vector.tensor_scalar_add(qTt[tau][:], p[:],
                                        bq16T[:, tau:tau + 1])

        def emit_k(tau, ch):
            # k bias dropped: q.bk is constant per query row, cancels in
            # the softmax
            sl = slice(ch * NQ, (ch + 1) * NQ)
            p = prjps.tile([P, NQ], F32, tag="prj", name='prj')
            for j in range(NJ):
                nc.tensor.matmul(
                    p[:], lhsT=wkv4[:, j, :, tau * P:(tau + 1) * P],
                    rhs=h3(j)[:, :, sl],
                    start=(j == 0), stop=(j == NJ - 1), perf_mode=DR)
            nc.vector.tensor_copy(kTt[tau][:, sl], p[:])

        def emit_v(nt, vg):
            # v bias folded host-side into bo_eff
            sl = slice(D + vg * NQ, D + (vg + 1) * NQ)
            p = prjps.tile([P, NQ], F32, tag="prj", name='prj')
            for j in range(NJ):
                nc.tensor.matmul(
                    p[:], lhsT=h3(j)[:, :, nt * P:(nt + 1) * P],
                    rhs=wkv4[:, j, :, sl],
                    start=(j == 0), stop=(j == NJ - 1), perf_mode=DR)
            pv = p.rearrange("p (h w) -> p h w", w=DH)
            nc.vector.tensor_copy(
                v4(nt // 2)[:, nt % 2, vg * 8:(vg + 1) * 8, 0:DH], pv[:])

        # phase 2 proper: q, k tiles 0-1 (heads 0-3), v group 0 (heads 0-7);
        # the rest is emitted inside the attention loop as PE filler so the
        # exp stream on ACT starts as early as possible
        for j in range(NJ):
            nc.vector.memset(v4(j)[:, :, :, DH:DH + 1], 1.0)
        for tau in range(KT):
            emit_q(tau)
        for tau in range(2):
            emit_k(tau, 0)
            emit_k(tau, 1)
        for nt in range(KT):
            emit_v(nt, 0)

        # Wo prefetch + the f32 own-half x reload (needed only at P4)
        woch = [wop.tile([P, D], BF16, tag=f"wo{k}", name=f"wo{k}")
                for k in range(KT)]
        for k in range(KT):
            chained_dma(woch[k][:], Wo[k * P:(k + 1) * P, :], e=0)
        for k in range(KT):
            chained_dma(xo[k][:], xoT[k * P:(k + 1) * P, :], e=0)

        # ---------------- phase 3: attention + filler ----------------
        with ExitStack() as ph:
            epool = ph.enter_context(tc.tile_pool(name="p3e", bufs=2))
            spool = ph.enter_context(tc.tile_pool(name="p3s", bufs=2))
            modw = ph.enter_context(tc.tile_pool(name="p3mw", bufs=12))
            ps_sim = ph.enter_context(
                tc.tile_pool(name="ps_sim", bufs=2, space="PSUM"))
            ps_o = ph.enter_context(
                tc.tile_pool(name="ps_o", bufs=2, space="PSUM"))

            mod_wch = {}

            def issue_mod_dma(gp):
                # 2 mod groups (cols (4+2gp)*NQ..) per call, 8 k-tile DMAs
                mod_wch[gp] = [modw.tile([P, 2 * NQ], BF16, tag="wadach",
                                         name='wadach') for _ in range(KT)]
                for k in range(KT):
                    di = nc.sync.dma_start(
                        mod_wch[gp][k][:],
                        Wada[k * P:(k + 1) * P,
                             (4 + 2 * gp) * NQ:(6 + 2 * gp) * NQ])
                    last_dma[0] = di

            def emit_mod_pair(gp):
                for gi in range(2):
                    g = 4 + 2 * gp + gi
                    mp = prjps.tile([P, NQ], F32, tag="prj", name='prj')
                    for k in range(KT):
                        nc.tensor.matmul(
                            mp[0:1, :], lhsT=csT[:, k:k + 1],
                            rhs=mod_wch[gp][k][:, gi * NQ:(gi + 1) * NQ],
                            start=(k == 0), stop=(k == KT - 1))
                    nc.vector.tensor_add(
                        mod_row[0:1, g * NQ:(g + 1) * NQ], mp[0:1, :],
                        bada_sb[0:1, g * NQ:(g + 1) * NQ])

            # filler schedule: k tiles must complete before their head
            # pairs; mod groups (needed only at P4+) fill the tail
            filler = {hh: [] for hh in range(KT)}
            for i, (tau, ch) in enumerate(
                    [(t, c) for t in range(2, KT) for c in range(2)]):
                filler[min(max(1, tau - 1 + (i % 2)), KT - 1)].append(
                    ('k', tau, ch))
            for nt in range(KT):
                filler[1 + (nt * 3) // KT].append(('v', nt, 1))
            for gp in range(4):
                filler[4 + gp].append(('mod', gp, 0))

            for hp in range(KT):       # head pairs
                if hp == 3:
                    issue_mod_dma(0)
                    issue_mod_dma(1)
                if hp == 5:
                    issue_mod_dma(2)
                if hp == 6:
                    issue_mod_dma(3)
                for kind, a0, a1 in filler[hp]:
                    if kind == 'k':
                        emit_k(a0, a1)
                    elif kind == 'v':
                        emit_v(a0, a1)
                    else:
                        emit_mod_pair(a0)

                # e for both heads of the pair, as fp8 key-pair slots:
                # layout [p, (j2 4, t 2, head 2, n 512)]
                et = epool.tile([P, 8 * 2 * NQ], FP8, tag="et", name='et')
                et5 = et.rearrange("p (j t h n) -> p j t h n", j=NJ, t=2,
                                   h=2)
                for kt in range(KT):
                    simps = ps_sim.tile([P, 2 * NQ], F32, tag="sim",
                                        name='sim')
                    for hi in range(2):
                        nc.tensor.matmul(
                            simps[:, hi * NQ:(hi + 1) * NQ],
                            lhsT=kTt[hp][hi * DH:(hi + 1) * DH,
                                         kt * P:(kt + 1) * P],
                            rhs=qTt[hp][hi * DH:(hi + 1) * DH, :],
                            start=True, stop=True)
                    nc.scalar.activation(
                        et[:, kt * 2 * NQ:(kt + 1) * 2 * NQ], simps[:],
                        AF.Exp, bias=nshift_t[:], scale=SSCALE)

                for hi in range(2):
                    h = 2 * hp + hi
                    pos = ps_o.tile([DH + 1, NQ], F32, tag="ov", name='ov')
                    for j2 in range(NJ):
                        nc.tensor.matmul(
                            pos[:], lhsT=v4(j2)[:, :, h, :],
                            rhs=et5[:, j2, :, hi, :],
                            start=(j2 == 0), stop=(j2 == NJ - 1),
                            perf_mode=DR)
                    den = spool.tile([1, NQ], F32, tag="den", name='den')
                    nc.vector.tensor_copy(den[:], pos[DH:DH + 1, :])
                    inv = spool.tile([1, NQ], F32, tag="inv", name='inv')
                    nc.vector.reciprocal_approx_fast(inv[:], den[:])
                    binv = spool.tile([DH, NQ], F32, tag="binv",
                                      name='binv')
                    nc.gpsimd.partition_broadcast(binv[:], inv[:])
                    nc.vector.tensor_mul(
                        outT[hp][hi * DH:(hi + 1) * DH, :],
                        pos[0:DH, :], binv[:])

        prj_cm.__exit__(None, None, None)
        wq_cm.__exit__(None, None, None)
        qkv_cm.__exit__(None, None, None)
        wkv_cm.__exit__(None, None, None)
        hp_cm.__exit__(None, None, None)

        # ---------------- phase 4: Wo + residual + ln2 + h2 ----------------
        # right-side creation order fixes pop order: w1p/w2p (root, pop
        # last), then g (pops after P6), x1 (P6), h2 (P5)
        w1p = root.enter_context(tc.tile_pool(name="w1p", bufs=2,
                                              side='right'))
        w2p = root.enter_context(tc.tile_pool(name="w2p", bufs=4,
                                              side='right'))
        w1tiles = {}
        W1C = KT * NQ   # 4096 cols per group

        def issue_w1_dma(grp):
            th = w1p.tile([P, W1C], BF16, tag="w1", name='w1')
            chained_dma(th[:], W1r[:, grp * W1C:(grp + 1) * W1C], e=0)
            w1tiles[grp] = th

        issue_w1_dma(0)
        issue_w1_dma(1)

        g_cm = tc.tile_pool(name="gp", bufs=1, side='right')
        gp_ = g_cm.__enter__()
        gTt = [gp_.tile([P, NQ], BF16, tag=f"g{m}", name=f"g{m}")
               for m in range(MT)]
        x1_cm = tc.tile_pool(name="x1p", bufs=1, side='right')
        x1p = x1_cm.__enter__()
        x1t = [x1p.tile([P, NQ], BF16, tag=f"x1{k}", name=f"x1{k}")
               for k in range(KT)]
        h2_cm = tc.tile_pool(name="h2p", bufs=1, side='right')
        h2p = h2_cm.__enter__()
        h2t = [h2p.tile([P, NQ], BF16, tag=f"h2{k}", name=f"h2{k}")
               for k in range(KT)]

        with ExitStack() as ph:
            rows4 = ph.enter_context(tc.tile_pool(name="p4rows", bufs=1))
            tpool = ph.enter_context(tc.tile_pool(name="p4t", bufs=2))

            with ExitStack() as sec:
                pscol2 = sec.enter_context(
                    tc.tile_pool(name="pscol2", bufs=1, space="PSUM"))

                # g_msa/16 columns from mod groups 4-5 (g_mlp is built
                # later, inside the ln2 rows chain where the PE idles)
                for dstT, base, sc in ((gmsaT, 4 * NQ, 1.0 / 16.0),):
                    psc2 = pscol2.tile([P, KT], F32, tag="colps2",
                                       name='colps2')
                    for j in range(KT):
                        nc.tensor.matmul(
                            psc2[:, j:j + 1],
                            lhsT=mod_row[0:1,
                                         base + j * P:base + (j + 1) * P],
                            rhs=ones_row[0:1, 0:1],
                            start=True, stop=True)
                    nc.vector.tensor_scalar_mul(dstT[:], psc2[:], sc)

            with ExitStack() as sec:
                psy = sec.enter_context(
                    tc.tile_pool(name="psy", bufs=2, space="PSUM"))
                psstat2 = sec.enter_context(
                    tc.tile_pool(name="psstat2", bufs=1, space="PSUM"))

                # Wo + residual, with the ln2 stat stream interleaved
                # per-tau so ACT/PE overlap
                ss = psstat2.tile([1, NQ], F32, tag="st2s", name='st2s')
                sq_ps = psstat2.tile([1, NQ], F32, tag="st2q", name='st2q')
                for tau in range(KT):
                    p = psy.tile([P, NQ], F32, tag="y1", name='y1')
                    for k in range(KT):
                        nc.tensor.matmul(
                            p[:], lhsT=woch[k][:, tau * P:(tau + 1) * P],
                            rhs=outT[k][:],
                            start=(k == 0), stop=(k == KT - 1))
                    tmp = tpool.tile([P, NQ], F32, tag="y1s", name='y1s')
                    nc.vector.tensor_scalar(tmp[:], p[:],
                                            bo16T[:, tau:tau + 1],
                                            gmsaT[:, tau:tau + 1],
                                            ALU.add, ALU.mult)
                    nc.vector.tensor_add(x1t[tau][:], xo[tau][:],
                                         tmp[:])
                    sq = tpool.tile([P, NQ], BF16, tag="x1sq", name='x1sq')
                    nc.scalar.activation(sq[:], x1t[tau][:], AF.Square)
                    nc.tensor.matmul(ss[:], lhsT=ones_col_b[:],
                                     rhs=x1t[tau][:],
                                     start=(tau == 0), stop=(tau == KT - 1))
                    nc.tensor.matmul(sq_ps[:], lhsT=ones_col_b[:],
                                     rhs=sq[:],
                                     start=(tau == 0), stop=(tau == KT - 1))
                mu2 = rows4.tile([1, NQ], F32, name='mu2')
                ex22 = rows4.tile([1, NQ], F32, name='ex22')
                nc.vector.tensor_scalar_mul(mu2[:], ss[:], 1.0 / D)
                nc.vector.tensor_scalar_mul(ex22[:], sq_ps[:], 1.0 / D)

            with ExitStack() as sec:
                psab2 = sec.enter_context(
                    tc.tile_pool(name="psab2", bufs=2, space="PSUM"))
                pscol3 = sec.enter_context(
                    tc.tile_pool(name="pscol3", bufs=1, space="PSUM"))
                psc3 = pscol3.tile([P, KT], F32, tag="colps3",
                                   name='colps3')
                for j in range(KT):
                    nc.tensor.matmul(
                        psc3[:, j:j + 1],
                        lhsT=mod_row[0:1,
                                     10 * NQ + j * P:10 * NQ + (j + 1) * P],
                        rhs=ones_row[0:1, 0:1],
                        start=True, stop=True)
                nc.vector.tensor_scalar_mul(gmlpT[:], psc3[:], 1.0)
                scr2 = rows4.tile([1, NQ], F32, name='scr2')
                nc.vector.tensor_mul(scr2[:], mu2[:], mu2[:])
                nc.vector.tensor_sub(ex22[:], ex22[:], scr2[:])
                nc.scalar.activation(scr2[:], ex22[:], AF.Sqrt,
                                     bias=eps_t[:])
                nc.vector.reciprocal_approx_fast(ex22[:], scr2[:])
                a2 = rows4.tile([1, NQ], BF16, name='a2')
                nc.vector.tensor_copy(a2[:], ex22[:])
                b2r = rows4.tile([1, NQ], BF16, name='b2r')
                nc.vector.scalar_tensor_tensor(b2r[:], mu2[:], -1.0,
                                               ex22[:], ALU.mult, ALU.mult)
                S2_row = rows4.tile([1, D], BF16, name='S2_row')
                nc.vector.tensor_scalar_add(S2_row[:],
                                            mod_row[0:1, 8 * NQ:10 * NQ],
                                            1.0)

                for k in range(KT):
                    pa = psab2.tile([P, NQ], F32, tag="pA2", name='pA2')
                    pb = psab2.tile([P, NQ], F32, tag="pB2", name='pB2')
                    nc.tensor.matmul(
                        pa[:], lhsT=S2_row[0:1, k * P:(k + 1) * P],
                        rhs=a2[:], start=True, stop=True)
                    nc.tensor.matmul(
                        pb[:], lhsT=S2_row[0:1, k * P:(k + 1) * P],
                        rhs=b2r[:], start=True, stop=False)
                    nc.tensor.matmul(
                        pb[:],
                        lhsT=mod_row[0:1, 6 * NQ + k * P:6 * NQ + (k + 1) * P],
                        rhs=ones_rowq[:], start=False, stop=True)
                    nc.vector.tensor_mul(h2t[k][:], x1t[k][:], pa[:])
                    nc.vector.tensor_add(h2t[k][:], h2t[k][:], pb[:])

        op_cm.__exit__(None, None, None)
        x_cm.__exit__(None, None, None)

        # ---------------- phase 5: W1 + gelu ----------------
        with ExitStack() as ph:
            ps1 = ph.enter_context(
                tc.tile_pool(name="ps1", bufs=4, space="PSUM"))

            for grp in range(GRP):
                if grp + 2 < GRP:
                    issue_w1_dma(grp + 2)
                th = w1tiles[grp]
                th3 = th.rearrange("p (k c) -> p k c", k=KT)
                for dot in range(4):
                    m = 4 * grp + dot
                    msl = slice(dot * P, (dot + 1) * P)
                    p = ps1.tile([P, NQ], F32, tag="m1", name='m1')
                    for k in range(KT):
                        nc.tensor.matmul(p[:], lhsT=th3[:, k, msl],
                                         rhs=h2t[k][:],
                                         start=(k == 0), stop=(k == KT - 1))
                    nc.scalar.activation(gTt[m][:], p[:],
                                         AF.Gelu_apprx_tanh,
                                         bias=b1T[:, m:m + 1])

        h2_cm.__exit__(None, None, None)

        # ---------------- phase 6: W2 (streamed) + output ----------------
        with ExitStack() as ph:
            opool = ph.enter_context(tc.tile_pool(name="p6o", bufs=3))
            ps2 = ph.enter_context(
                tc.tile_pool(name="ps2", bufs=1, space="PSUM"))

            for half in range(2):
                pacc = [ps2.tile([P, NQ], F32, tag=f"acc{d}",
                                 name=f"acc{d}") for d in range(4)]
                for mj in range(MT // 4):
                    base = (half * MT + 4 * mj) * NQ
                    w2c = w2p.tile([P, 4 * NQ], BF16, tag="w2", name='w2')
                    chained_dma(w2c[:], W2r[:, base:base + 4 * NQ],
                                e=(mj % 2))
                    w23 = w2c.rearrange("p (u c) -> p u c", u=4)
                    for u in range(4):
                        mk = 4 * mj + u
                        for d in range(4):
                            nc.tensor.matmul(
                                pacc[d][:],
                                lhsT=w23[:, u, d * P:(d + 1) * P],
                                rhs=gTt[mk][:],
                                start=(mk == 0), stop=(mk == MT - 1))
                for d in range(4):
                    tau = half * 4 + d
                    tmp = opool.tile([P, NQ], F32, tag="m2s", name='m2s')
                    nc.vector.tensor_scalar(tmp[:], pacc[d][:],
                                            b2T[:, tau:tau + 1],
                                            gmlpT[:, tau:tau + 1],
                                            ALU.add, ALU.mult)
                    yt = opool.tile([P, NQ], F32, tag="yout", name='yout')
                    nc.vector.tensor_add(yt[:], x1t[tau][:], tmp[:])
                    nc.sync.dma_start(yT[tau * P:(tau + 1) * P, :], yt[:])

        x1_cm.__exit__(None, None, None)
        g_cm.__exit__(None, None, None)

    nc.compile()
    return nc


_NC = None


def _get_nc():
    global _NC
    if _NC is None:
        _NC = build()
    return _NC


def _prep_inputs(x, c, Wq, bq, Wkv, bkv, Wo, bo, W1, b1, W2, b2, Wada, bada):
    import ml_dtypes
    f = np.float32
    bf = ml_dtypes.bfloat16
    f8 = ml_dtypes.float8_e4m3
    # (bf used for xbT below)

    def q8(v):
        return np.clip(np.asarray(v, f), -240.0, 240.0).astype(f8)

    def drp(W):
        # [K, M] -> [128, J*2*M] partition-major DoubleRow layout:
        # col j*(2M) + t*M + m  <->  W[(2j+t)*128 + p, m]
        K, M = W.shape
        J = K // (2 * P)
        return np.ascontiguousarray(
            W.reshape(J, 2, P, M).transpose(2, 0, 1, 3).reshape(P, J * 2 * M))

    def pgrp(W, ncol):
        # bf16 [K, M] -> [128, (g, k, ncol)]: col g*(K//128*ncol) + k*ncol+cc
        K, M = W.shape
        KTl = K // P
        G = M // ncol
        return np.ascontiguousarray(
            W.reshape(KTl, P, G, ncol).transpose(1, 2, 0, 3)
            .reshape(P, K * M // P))

    Wq8 = drp(q8(np.asarray(Wq, f) * 16.0))
    Wkv8 = drp(q8(np.asarray(Wkv, f) * 16.0))

    col = lambda v, n: np.ascontiguousarray(np.asarray(v, f).reshape(n, P).T)

    bo_eff = np.asarray(bkv, f)[D:] @ np.asarray(Wo, f) + np.asarray(bo, f)

    # W2 [MLPD, D] -> [128, (half, mk, 512)]
    W2f = np.asarray(W2, f).astype(bf)
    W2r = np.ascontiguousarray(
        W2f.reshape(MT, P, 2, NQ).transpose(1, 2, 0, 3).reshape(P, -1))

    shared = {
        "Wqd": Wq8, "Wkvd": Wkv8,
        "W1r": pgrp(np.asarray(W1, f).astype(bf), NQ),
        "W2r": W2r,
        "Wo": np.asarray(Wo, f).astype(bf),
        "Wada": np.asarray(Wada, f).astype(bf),
        "bada_r": np.asarray(bada, f).reshape(1, -1).astype(bf),
        "bq16_c": col(np.asarray(bq, f) * 16.0, KT),
        "bo16_c": col(bo_eff * 16.0, KT),
        "b1_c": col(b1, MT),
        "b2_c": col(b2, KT),
    }
    in_maps = []
    for core in range(NCORES):
        b, half = core // 2, core % 2
        xb = np.asarray(x[b], np.float32)
        perm_x = np.concatenate(
            [xb[half * NQ:(half + 1) * NQ],
             xb[(1 - half) * NQ:(2 - half) * NQ]], axis=0)
        m = dict(shared)
        m["xbT"] = np.ascontiguousarray(perm_x.T).astype(bf)
        m["xoT"] = np.ascontiguousarray(perm_x[0:NQ].T)
        m["crow"] = np.asarray(c[b:b + 1], np.float32)
        in_maps.append(m)
    return in_maps


def _run(inputs, trace=False):
    nc = _get_nc()
    in_maps = _prep_inputs(**inputs)
    res = run_bass_kernel_spmd(nc, in_maps, core_ids=list(range(NCORES)),
                               trace=trace)
    B = 4
    y = np.empty((B, N, D), np.float32)
    for core in range(NCORES):
        b, half = core // 2, core % 2
        y[b, half * NQ:(half + 1) * NQ, :] = res.results[core]["yT"].T
    return y, res


def kernel(**inputs):
    y, _ = _run(inputs, trace=False)
    return y


# revision 77
# speedup vs baseline: 1.0140x; 1.0140x over previous
"""AdaLN attention block (DiT-style) on 8 TRN2 NeuronCores.

Sharding: 8 cores = 4 batches x 2 token-halves, no collectives. Core c handles
batch c//2 and query-token half c%2: layernorm1 and k/v are computed over the
full (permuted) sequence, everything else only for the own 512 query rows.

Cost-model reality (measured): matmul time = out_free_size x 1 cycle
regardless of dtype (DoubleRow gets no 0.5 discount; f32 pays 4x), so the
only PE lever is matmul COUNT. fp8 DoubleRow still halves the count for
K-contractions (K=256 per matmul):
- q/k/v projections: fp8 h (x1) against fp8 weights (x16), 1-term DR -> half
  the matmuls of bf16. q/k evicted to bf16 x16 tiles; sim runs bf16 (K=64,
  same cost as fp8 here, better numerics).
- attn@v contracts key tiles as DR pairs: exp is written by ACT directly as
  fp8 into key-pair slots (exp(sim*s - 6) fits e4m3's 240 max), v fp8 x16
  with an exact fp8 ones-column feeding the softmax denominator; 1/den is
  partition-broadcast on GPSIMD (no f32 matmul, no PSUM copies).
- LN stats contract via fp8 DoubleRow pairs of bf16->fp8 x/x^2 copies.
- MLP and Wo stay bf16 (3-term fp8 would cost MORE matmuls than bf16).
- Bias algebra folded host-side: bk dropped (cancels in softmax), bv folded
  into bo_eff = bv@Wo + bo.
- Weights are re-laid-out host-side to [128, ...] partition-major so each
  stream is a few large contiguous DMAs.
"""

import numpy as np
from contextlib import ExitStack

import concourse.bass as bass
import concourse.bacc as bacc
import concourse.mybir as mybir
from concourse import tile
from concourse.tile import add_dep_helper
from concourse.bass_utils import run_bass_kernel_spmd

P = 128
D = 1024
N = 1024
NQ = 512
H = 16
DH = 64
MLPD = 4096
EPS = 1e-6
NCORES = 8
SHIFT = 6.0                      # softmax shift so exp fits e4m3 (max 240)
SSCALE = (DH ** -0.5) / 256.0    # q16.k16 psum -> sim

F32 = mybir.dt.float32
BF16 = mybir.dt.bfloat16
FP8 = mybir.dt.float8e4
AF = mybir.ActivationFunctionType
ALU = mybir.AluOpType
DR = mybir.MatmulPerfMode.DoubleRow

KT = D // P            # 8 contraction tiles over D
NJ = KT // 2           # 4 k-tile pairs
MT = MLPD // P         # 32 tiles over MLP dim
GRP = MLPD // NQ       # 8 MLP column groups


def build():
    nc = bacc.Bacc("TRN2", target_bir_lowering=False, debug=False,
                   num_devices=NCORES)

    xbT = nc.dram_tensor("xbT", [D, N], BF16, kind="ExternalInput")
    xoT = nc.dram_tensor("xoT", [D, NQ], F32, kind="ExternalInput")
    crow = nc.dram_tensor("crow", [1, D], F32, kind="ExternalInput")
    # fp8 projection weights (x16) in [128, ...] partition-major DR layout
    Wqd = nc.dram_tensor("Wqd", [P, NJ * 2 * D], FP8, kind="ExternalInput")
    Wkvd = nc.dram_tensor("Wkvd", [P, NJ * 4 * D], FP8,
                          kind="ExternalInput")
    Wo = nc.dram_tensor("Wo", [D, D], BF16, kind="ExternalInput")
    # W1 bf16 [128, (grp, k, c)]; W2 bf16 [128, (half, mk, c)]
    W1r = nc.dram_tensor("W1r", [P, D * MLPD // P], BF16,
                         kind="ExternalInput")
    W2r = nc.dram_tensor("W2r", [P, MLPD * D // P], BF16,
                         kind="ExternalInput")
    Wada = nc.dram_tensor("Wada", [D, 6 * D], BF16, kind="ExternalInput")
    bada_r = nc.dram_tensor("bada_r", [1, 6 * D], BF16, kind="ExternalInput")
    bq16_c = nc.dram_tensor("bq16_c", [P, KT], F32, kind="ExternalInput")
    bo16_c = nc.dram_tensor("bo16_c", [P, KT], F32, kind="ExternalInput")
    b1_c = nc.dram_tensor("b1_c", [P, MT], F32, kind="ExternalInput")
    b2_c = nc.dram_tensor("b2_c", [P, KT], F32, kind="ExternalInput")
    yT = nc.dram_tensor("yT", [D, NQ], F32, kind="ExternalOutput")

    with tile.TileContext(nc) as tc, ExitStack() as root:
        const = root.enter_context(tc.tile_pool(name="const", bufs=1))
        rootrows = root.enter_context(tc.tile_pool(name="rootrows", bufs=1))

        # c row first: the silu/csT chain gates the mod computation
        c_sb = rootrows.tile([1, D], F32, name='c_sb')
        nc.sync.dma_start(c_sb[:], crow[:])

        ones_col_b = const.tile([P, 1], BF16, name='ones_col_b')
        nc.vector.memset(ones_col_b[:], 1.0)
        ones_row = const.tile([1, 1], BF16, name='ones_row')
        nc.vector.memset(ones_row[:], 1.0)
        ones_rowq = const.tile([1, NQ], BF16, name='ones_rowq')
        nc.vector.memset(ones_rowq[:], 1.0)
        eps_t = const.tile([1, 1], F32, name='eps_t')
        nc.vector.memset(eps_t[:], EPS)
        nshift_t = const.tile([P, 1], F32, name='nshift_t')
        nc.vector.memset(nshift_t[:], -SHIFT)

        bq16T = const.tile([P, KT], F32, name='bq16T')
        bo16T = const.tile([P, KT], F32, name='bo16T')
        b1T = const.tile([P, MT], F32, name='b1T')
        b2T = const.tile([P, KT], F32, name='b2T')
        nc.gpsimd.dma_start(bq16T[:], bq16_c[:])
        nc.gpsimd.dma_start(bo16T[:], bo16_c[:])
        nc.gpsimd.dma_start(b1T[:], b1_c[:])
        nc.gpsimd.dma_start(b2T[:], b2_c[:])

        csT = const.tile([P, KT], BF16, name='csT')
        gmsaT = const.tile([P, KT], F32, name='gmsaT')   # g_msa / 16
        gmlpT = const.tile([P, KT], F32, name='gmlpT')   # g_mlp

        # left-side persistent pools; pop order: hp (P3), op (P4), x (P4)
        x_cm = tc.tile_pool(name="xp", bufs=1, side='left')
        xp_ = x_cm.__enter__()
        xt = [xp_.tile([P, N], BF16, tag=f"x{k}", name=f"x{k}")
              for k in range(KT)]
        xo = [xp_.tile([P, NQ], F32, tag=f"xo{k}", name=f"xo{k}")
              for k in range(KT)]
        for k in range(KT):
            nc.sync.dma_start(xt[k][:], xbT[k * P:(k + 1) * P, :])

        op_cm = tc.tile_pool(name="op", bufs=1, side='left')
        op_ = op_cm.__enter__()
        outT = [op_.tile([P, NQ], BF16, tag=f"o{k}", name=f"o{k}")
                for k in range(KT)]

        hp_cm = tc.tile_pool(name="hp", bufs=1, side='left')
        hp_ = hp_cm.__enter__()
        # h as fp8 k-tile pairs: tile j = k-tiles (2j: cols 0:N, 2j+1: N:2N)
        hpr = [hp_.tile([P, 2 * N], FP8, tag=f"h{j}", name=f"h{j}")
               for j in range(NJ)]

        mod_row = rootrows.tile([1, 6 * D], BF16, name='mod_row')

        # right-side pools; wop (root) at the bottom, the rest popped
        # after P2/P3 in reverse creation order
        wop = root.enter_context(tc.tile_pool(name="wop", bufs=1,
                                              side='right'))
        wkv_cm = tc.tile_pool(name="wkvp", bufs=1, side='right')
        wkvp = wkv_cm.__enter__()
        wkv_t = wkvp.tile([P, NJ * 4 * D], FP8, name='wkv_t')
        HKV = NJ * 2 * D
        wkv4 = wkv_t.rearrange("p (j t c) -> p j t c", j=NJ, t=2)

        qkv_cm = tc.tile_pool(name="qkvp", bufs=1, side='right')
        qkvp = qkv_cm.__enter__()
        bada_sb = qkvp.tile([1, 6 * D], BF16, name='bada_sb')
        nc.gpsimd.dma_start(bada_sb[:], bada_r[:])

        def h3(j):
            return hpr[j].rearrange("p (t n) -> p t n", t=2)

        last_dma = {0: None, 1: None}

        def chained_dma(dst, src_ap, e=None):
            if e is None:
                e = chained_dma.flip
                chained_dma.flip ^= 1
            eng = nc.sync if e == 0 else nc.gpsimd
            di = eng.dma_start(dst, src_ap)
            if last_dma[e] is not None:
                add_dep_helper(di.ins, last_dma[e].ins, sync=True,
                               reason="prefetch ordered after critical DMAs")
            last_dma[e] = di
            return di
        chained_dma.flip = 0

        # ---------------- phase 0: mod groups 0-3 & ln1 stats ---------------
        p0_cm = tc.tile_pool(name="p0rows", bufs=1)
        rows = p0_cm.__enter__()
        sq_cm = tc.tile_pool(name="p0sq", bufs=2)
        sqpool = sq_cm.__enter__()

        with ExitStack() as sec:
            wpool = sec.enter_context(tc.tile_pool(name="p0w", bufs=4))
            pscol = sec.enter_context(
                tc.tile_pool(name="pscol", bufs=1, space="PSUM"))
            psmod = sec.enter_context(
                tc.tile_pool(name="psmod", bufs=1, space="PSUM"))
            psstat = sec.enter_context(
                tc.tile_pool(name="psstat", bufs=1, space="PSUM"))

            # silu(c) and its column layout
            cs_row = rows.tile([1, D], BF16, name='cs_row')
            nc.scalar.activation(cs_row[:], c_sb[:], AF.Silu)
            psc = pscol.tile([P, KT], F32, tag="colps", name='colps')
            for j in range(KT):
                nc.tensor.matmul(psc[:, j:j + 1],
                                 lhsT=cs_row[0:1, j * P:(j + 1) * P],
                                 rhs=ones_row[0:1, 0:1],
                                 start=True, stop=True)
            nc.vector.tensor_copy(csT[:], psc[:])

            # mod groups 0-3 = sh_msa | sc_msa
            mps = [psmod.tile([1, NQ], F32, tag=f"modps{g}",
                              name=f"modps{g}") for g in range(4)]
            for k in range(KT):
                wch4 = wpool.tile([P, 4 * NQ], BF16, tag="wada",
                                  name='wada')
                di = nc.gpsimd.dma_start(wch4[:],
                                         Wada[k * P:(k + 1) * P, 0:4 * NQ])
                last_dma[1] = di
                for g in range(4):
                    nc.tensor.matmul(
                        mps[g][:], lhsT=csT[:, k:k + 1],
                        rhs=wch4[:, g * NQ:(g + 1) * NQ],
                        start=(k == 0), stop=(k == KT - 1))
            for g in range(4):
                nc.vector.tensor_add(
                    mod_row[0:1, g * NQ:(g + 1) * NQ], mps[g][:],
                    bada_sb[0:1, g * NQ:(g + 1) * NQ])

            # ln1 stats: sum and sumsq over d via bf16 x copies (ACT)
            mu_row = rows.tile([1, N], F32, name='mu_row')
            ex2_row = rows.tile([1, N], F32, name='ex2_row')
            for ch in range(2):
                sl = slice(ch * NQ, (ch + 1) * NQ)
                ss = psstat.tile([1, NQ], F32, tag="st_s", name='st_s')
                sq_ps = psstat.tile([1, NQ], F32, tag="st_q", name='st_q')
                for k in range(KT):
                    sq = sqpool.tile([P, NQ], BF16, tag="xsq", name='xsq')
                    nc.scalar.activation(sq[:], xt[k][:, sl], AF.Square)
                    nc.tensor.matmul(ss[:], lhsT=ones_col_b[:],
                                     rhs=xt[k][:, sl],
                                     start=(k == 0), stop=(k == KT - 1))
                    nc.tensor.matmul(sq_ps[:], lhsT=ones_col_b[:],
                                     rhs=sq[:],
                                     start=(k == 0), stop=(k == KT - 1))
                nc.vector.tensor_scalar_mul(mu_row[0:1, sl], ss[:], 1.0 / D)
                nc.vector.tensor_scalar_mul(ex2_row[0:1, sl], sq_ps[:],
                                            1.0 / D)

            scr_row = rows.tile([1, N], F32, name='scr_row')
            a_row = rows.tile([1, N], BF16, name='a_row')
            b_row = rows.tile([1, N], BF16, name='b_row')
            S1_row = rows.tile([1, D], BF16, name='S1_row')
            nc.vector.tensor_scalar_add(S1_row[:], mod_row[0:1, D:2 * D],
                                        1.0)
            for ch in range(2):
                sl = slice(ch * NQ, (ch + 1) * NQ)
                nc.vector.tensor_mul(scr_row[0:1, sl], mu_row[0:1, sl],
                                     mu_row[0:1, sl])
                nc.vector.tensor_sub(ex2_row[0:1, sl], ex2_row[0:1, sl],
                                     scr_row[0:1, sl])
                nc.scalar.activation(scr_row[0:1, sl], ex2_row[0:1, sl],
                                     AF.Sqrt, bias=eps_t[:])
                nc.vector.reciprocal_approx_fast(ex2_row[0:1, sl],
                                                 scr_row[0:1, sl])
                nc.vector.tensor_copy(a_row[0:1, sl], ex2_row[0:1, sl])
                nc.vector.scalar_tensor_tensor(b_row[0:1, sl],
                                               mu_row[0:1, sl], -1.0,
                                               ex2_row[0:1, sl],
                                               ALU.mult, ALU.mult)

        # q weights on top of the right stack (popped after P3)
        wq_cm = tc.tile_pool(name="wqp", bufs=1, side='right')
        wqp = wq_cm.__enter__()
        wq_t = wqp.tile([P, NJ * 2 * D], FP8, name='wq_t')
        nc.scalar.dma_start(wq_t[:], Wqd[:])
        nc.scalar.dma_start(wkv_t[:, 0:HKV], Wkvd[:, 0:HKV])
        nc.scalar.dma_start(wkv_t[:, HKV:2 * HKV], Wkvd[:, HKV:2 * HKV])
        wq4 = wq_t.rearrange("p (j t c) -> p j t c", j=NJ, t=2)

        # ------------- phase 1: h = fp8(x*A + B) ----------------
        with ExitStack() as sec:
            psab = sec.enter_context(
                tc.tile_pool(name="psab", bufs=2, space="PSUM"))
            for k in range(KT):
                for ch in range(2):
                    sl = slice(ch * NQ, (ch + 1) * NQ)
                    pa = psab.tile([P, NQ], F32, tag="pA", name='pA')
                    pb = psab.tile([P, NQ], F32, tag="pB", name='pB')
                    nc.tensor.matmul(
                        pa[:], lhsT=S1_row[0:1, k * P:(k + 1) * P],
                        rhs=a_row[0:1, sl], start=True, stop=True)
                    nc.tensor.matmul(
                        pb[:], lhsT=S1_row[0:1, k * P:(k + 1) * P],
                        rhs=b_row[0:1, sl], start=True, stop=False)
                    nc.tensor.matmul(
                        pb[:], lhsT=mod_row[0:1, k * P:(k + 1) * P],
                        rhs=ones_rowq[:], start=False, stop=True)
                    htmp = sqpool.tile([P, NQ], F32, tag="htmp",
                                       name='htmp')
                    nc.vector.tensor_mul(htmp[:], xt[k][:, sl], pa[:])
                    base = (k % 2) * N
                    nc.vector.tensor_add(
                        hpr[k // 2][:, base + ch * NQ:base + (ch + 1) * NQ],
                        htmp[:], pb[:])

        sq_cm.__exit__(None, None, None)
        p0_cm.__exit__(None, None, None)

        # ---------------- phase 2/3 shared: q/k/v emission ----------------
        # q/k bf16 x16 feature-major tiles (2 heads per tile); v fp8 x16
        # transposed (keys on partitions) as key-pair tiles with a fp8
        # ones column per head feeding the softmax denominator
        qTt = [qkvp.tile([P, NQ], BF16, tag=f"q{i}", name=f"q{i}")
               for i in range(KT)]
        kTt = [qkvp.tile([P, N], BF16, tag=f"k{i}", name=f"k{i}")
               for i in range(KT)]
        vpr = [qkvp.tile([P, 2 * H * (DH + 1)], FP8, tag=f"v{j}",
                         name=f"v{j}") for j in range(NJ)]

        prj_cm = tc.tile_pool(name="prjps", bufs=2, space="PSUM",
                              side='right')
        prjps = prj_cm.__enter__()

        def v4(j):
            return vpr[j].rearrange("p (t h w) -> p t h w", t=2, w=DH + 1)

        def emit_q(tau):
            p = prjps.tile([P, NQ], F32, tag="prj", name='prj')
            for j in range(NJ):
                nc.tensor.matmul(
                    p[:], lhsT=wq4[:, j, :, tau * P:(tau + 1) * P],
                    rhs=h3(j)[:, :, 0:NQ],
                    start=(j == 0), stop=(j == NJ - 1), perf_mode=DR)
            nc.vector.tensor_scalar_add(qTt[tau][:], p[:],
                                        bq16T[:, tau:tau + 1])

        def emit_k(tau, ch):
            # k bias dropped: q.bk is constant per query row, cancels in
            # the softmax
            sl = slice(ch * NQ, (ch + 1) * NQ)
            p = prjps.tile([P, NQ], F32, tag="prj", name='prj')
            for j in range(NJ):
                nc.tensor.matmul(
                    p[:], lhsT=wkv4[:, j, :, tau * P:(tau + 1) * P],
                    rhs=h3(j)[:, :, sl],
                    start=(j == 0), stop=(j == NJ - 1), perf_mode=DR)
            nc.vector.tensor_copy(kTt[tau][:, sl], p[:])

        def emit_v(nt, vg):
            # v bias folded host-side into bo_eff
            sl = slice(D + vg * NQ, D + (vg + 1) * NQ)
            p = prjps.tile([P, NQ], F32, tag="prj", name='prj')
            for j in range(NJ):
                nc.tensor.matmul(
                    p[:], lhsT=h3(j)[:, :, nt * P:(nt + 1) * P],
                    rhs=wkv4[:, j, :, sl],
                    start=(j == 0), stop=(j == NJ - 1), perf_mode=DR)
            pv = p.rearrange("p (h w) -> p h w", w=DH)
            nc.vector.tensor_copy(
                v4(nt // 2)[:, nt % 2, vg * 8:(vg + 1) * 8, 0:DH], pv[:])

        # phase 2 proper: q, k tiles 0-1 (heads 0-3), v group 0 (heads 0-7);
        # the rest is emitted inside the attention loop as PE filler so the
        # exp stream on ACT starts as early as possible
        for j in range(NJ):
            nc.vector.memset(v4(j)[:, :, :, DH:DH + 1], 1.0)
        for tau in range(KT):
            emit_q(tau)
        for tau in range(2):
            emit_k(tau, 0)
            emit_k(tau, 1)
        for nt in range(KT):
            emit_v(nt, 0)

        # Wo prefetch + the f32 own-half x reload (needed only at P4)
        woch = [wop.tile([P, D], BF16, tag=f"wo{k}", name=f"wo{k}")
                for k in range(KT)]
        for k in range(KT):
            chained_dma(woch[k][:], Wo[k * P:(k + 1) * P, :])
        for k in range(KT):
            chained_dma(xo[k][:], xoT[k * P:(k + 1) * P, :])

        # ---------------- phase 3: attention + filler ----------------
        with ExitStack() as ph:
            epool = ph.enter_context(tc.tile_pool(name="p3e", bufs=3))
            spool = ph.enter_context(tc.tile_pool(name="p3s", bufs=2))
            modw = ph.enter_context(tc.tile_pool(name="p3mw", bufs=9))
            ps_sim = ph.enter_context(
                tc.tile_pool(name="ps_sim", bufs=2, space="PSUM"))
            ps_o = ph.enter_context(
                tc.tile_pool(name="ps_o", bufs=2, space="PSUM"))

            mod_wch = {}

            def issue_mod_dma(gp):
                # 2 mod groups (cols (4+2gp)*NQ..) per call, 8 k-tile DMAs
                mod_wch[gp] = [modw.tile([P, 2 * NQ], BF16, tag="wadach",
                                         name='wadach') for _ in range(KT)]
                for k in range(KT):
                    di = nc.sync.dma_start(
                        mod_wch[gp][k][:],
                        Wada[k * P:(k + 1) * P,
                             (4 + 2 * gp) * NQ:(6 + 2 * gp) * NQ])
                    last_dma[0] = di

            def emit_mod_pair(gp):
                for gi in range(2):
                    g = 4 + 2 * gp + gi
                    mp = prjps.tile([P, NQ], F32, tag="prj", name='prj')
                    for k in range(KT):
                        nc.tensor.matmul(
                            mp[0:1, :], lhsT=csT[:, k:k + 1],
                            rhs=mod_wch[gp][k][:, gi * NQ:(gi + 1) * NQ],
                            start=(k == 0), stop=(k == KT - 1))
                    nc.vector.tensor_add(
                        mod_row[0:1, g * NQ:(g + 1) * NQ], mp[0:1, :],
                        bada_sb[0:1, g * NQ:(g + 1) * NQ])

            # filler schedule: k tiles must complete before their head
            # pairs; mod groups (needed only at P4+) fill the tail
            filler = {hh: [] for hh in range(KT)}
            for i, (tau, ch) in enumerate(
                    [(t, c) for t in range(2, KT) for c in range(2)]):
                filler[min(max(1, tau - 1 + (i % 2)), KT - 1)].append(
                    ('k', tau, ch))
            for nt in range(KT):
                filler[1 + (nt * 3) // KT].append(('v', nt, 1))
            for gp in range(4):
                filler[4 + gp].append(('mod', gp, 0))

            for hp in range(KT):       # head pairs
                if hp == 3:
                    issue_mod_dma(0)
                    issue_mod_dma(1)
                if hp == 5:
                    issue_mod_dma(2)
                if hp == 6:
                    issue_mod_dma(3)
                for kind, a0, a1 in filler[hp]:
                    if kind == 'k':
                        emit_k(a0, a1)
                    elif kind == 'v':
                        emit_v(a0, a1)
                    else:
                        emit_mod_pair(a0)

                # e for both heads of the pair, as fp8 key-pair slots:
                # layout [p, (j2 4, t 2, head 2, n 512)]
                et = epool.tile([P, 8 * 2 * NQ], FP8, tag="et", name='et')
                et5 = et.rearrange("p (j t h n) -> p j t h n", j=NJ, t=2,
                                   h=2)
                for kt in range(KT):
                    simps = ps_sim.tile([P, 2 * NQ], F32, tag="sim",
                                        name='sim')
                    for hi in range(2):
                        nc.tensor.matmul(
                            simps[:, hi * NQ:(hi + 1) * NQ],
                            lhsT=kTt[hp][hi * DH:(hi + 1) * DH,
                                         kt * P:(kt + 1) * P],
                            rhs=qTt[hp][hi * DH:(hi + 1) * DH, :],
                            start=True, stop=True)
                    nc.scalar.activation(
                        et[:, kt * 2 * NQ:(kt + 1) * 2 * NQ], simps[:],
                        AF.Exp, bias=nshift_t[:], scale=SSCALE)

                for hi in range(2):
                    h = 2 * hp + hi
                    pos = ps_o.tile([DH + 1, NQ], F32, tag="ov", name='ov')
                    for j2 in range(NJ):
                        nc.tensor.matmul(
                            pos[:], lhsT=v4(j2)[:, :, h, :],
                            rhs=et5[:, j2, :, hi, :],
                            start=(j2 == 0), stop=(j2 == NJ - 1),
                            perf_mode=DR)
                    den = spool.tile([1, NQ], F32, tag="den", name='den')
                    nc.vector.tensor_copy(den[:], pos[DH:DH + 1, :])
                    inv = spool.tile([1, NQ], F32, tag="inv", name='inv')
                    nc.vector.reciprocal_approx_fast(inv[:], den[:])
                    binv = spool.tile([DH, NQ], F32, tag="binv",
                                      name='binv')
                    nc.gpsimd.partition_broadcast(binv[:], inv[:])
                    nc.vector.tensor_mul(
                        outT[hp][hi * DH:(hi + 1) * DH, :],
                        pos[0:DH, :], binv[:])

        prj_cm.__exit__(None, None, None)
        wq_cm.__exit__(None, None, None)
        qkv_cm.__exit__(None, None, None)
        wkv_cm.__exit__(None, None, None)
        hp_cm.__exit__(None, None, None)

        # ---------------- phase 4: Wo + residual + ln2 + h2 ----------------
        # right-side creation order fixes pop order: w1p/w2p (root, pop
        # last), then g (pops after P6), x1 (P6), h2 (P5)
        w1p = root.enter_context(tc.tile_pool(name="w1p", bufs=2,
                                              side='right'))
        w2p = root.enter_context(tc.tile_pool(name="w2p", bufs=4,
                                              side='right'))
        w1tiles = {}
        W1C = KT * NQ   # 4096 cols per group

        def issue_w1_dma(grp):
            th = w1p.tile([P, W1C], BF16, tag="w1", name='w1')
            chained_dma(th[:], W1r[:, grp * W1C:(grp + 1) * W1C], e=0)
            w1tiles[grp] = th

        issue_w1_dma(0)
        issue_w1_dma(1)

        g_cm = tc.tile_pool(name="gp", bufs=1, side='right')
        gp_ = g_cm.__enter__()
        gTt = [gp_.tile([P, NQ], BF16, tag=f"g{m}", name=f"g{m}")
               for m in range(MT)]
        x1_cm = tc.tile_pool(name="x1p", bufs=1, side='right')
        x1p = x1_cm.__enter__()
        x1t = [x1p.tile([P, NQ], BF16, tag=f"x1{k}", name=f"x1{k}")
               for k in range(KT)]
        h2_cm = tc.tile_pool(name="h2p", bufs=1, side='right')
        h2p = h2_cm.__enter__()
        h2t = [h2p.tile([P, NQ], BF16, tag=f"h2{k}", name=f"h2{k}")
               for k in range(KT)]

        with ExitStack() as ph:
            rows4 = ph.enter_context(tc.tile_pool(name="p4rows", bufs=1))
            tpool = ph.enter_context(tc.tile_pool(name="p4t", bufs=2))

            with ExitStack() as sec:
                pscol2 = sec.enter_context(
                    tc.tile_pool(name="pscol2", bufs=1, space="PSUM"))

                # g_msa/16 columns from mod groups 4-5 (g_mlp is built
                # later, inside the ln2 rows chain where the PE idles)
                for dstT, base, sc in ((gmsaT, 4 * NQ, 1.0 / 16.0),):
                    psc2 = pscol2.tile([P, KT], F32, tag="colps2",
                                       name='colps2')
                    for j in range(KT):
                        nc.tensor.matmul(
                            psc2[:, j:j + 1],
                            lhsT=mod_row[0:1,
                                         base + j * P:base + (j + 1) * P],
                            rhs=ones_row[0:1, 0:1],
                            start=True, stop=True)
                    nc.vector.tensor_scalar_mul(dstT[:], psc2[:], sc)

            with ExitStack() as sec:
                psy = sec.enter_context(
                    tc.tile_pool(name="psy", bufs=2, space="PSUM"))
                psstat2 = sec.enter_context(
                    tc.tile_pool(name="psstat2", bufs=1, space="PSUM"))

                # Wo + residual, with the ln2 stat stream interleaved
                # per-tau so ACT/PE overlap
                ss = psstat2.tile([1, NQ], F32, tag="st2s", name='st2s')
                sq_ps = psstat2.tile([1, NQ], F32, tag="st2q", name='st2q')
                for tau in range(KT):
                    p = psy.tile([P, NQ], F32, tag="y1", name='y1')
                    for k in range(KT):
                        nc.tensor.matmul(
                            p[:], lhsT=woch[k][:, tau * P:(tau + 1) * P],
                            rhs=outT[k][:],
                            start=(k == 0), stop=(k == KT - 1))
                    tmp = tpool.tile([P, NQ], F32, tag="y1s", name='y1s')
                    nc.vector.tensor_scalar(tmp[:], p[:],
                                            bo16T[:, tau:tau + 1],
                                            gmsaT[:, tau:tau + 1],
                                            ALU.add, ALU.mult)
                    nc.vector.tensor_add(x1t[tau][:], xo[tau][:],
                                         tmp[:])
                    sq = tpool.tile([P, NQ], BF16, tag="x1sq", name='x1sq')
                    nc.scalar.activation(sq[:], x1t[tau][:], AF.Square)
                    nc.tensor.matmul(ss[:], lhsT=ones_col_b[:],
                                     rhs=x1t[tau][:],
                                     start=(tau == 0), stop=(tau == KT - 1))
                    nc.tensor.matmul(sq_ps[:], lhsT=ones_col_b[:],
                                     rhs=sq[:],
                                     start=(tau == 0), stop=(tau == KT - 1))
                mu2 = rows4.tile([1, NQ], F32, name='mu2')
                ex22 = rows4.tile([1, NQ], F32, name='ex22')
                nc.vector.tensor_scalar_mul(mu2[:], ss[:], 1.0 / D)
                nc.vector.tensor_scalar_mul(ex22[:], sq_ps[:], 1.0 / D)

            with ExitStack() as sec:
                psab2 = sec.enter_context(
                    tc.tile_pool(name="psab2", bufs=2, space="PSUM"))
                pscol3 = sec.enter_context(
                    tc.tile_pool(name="pscol3", bufs=1, space="PSUM"))
                psc3 = pscol3.tile([P, KT], F32, tag="colps3",
                                   name='colps3')
                for j in range(KT):
                    nc.tensor.matmul(
                        psc3[:, j:j + 1],
                        lhsT=mod_row[0:1,
                                     10 * NQ + j * P:10 * NQ + (j + 1) * P],
                        rhs=ones_row[0:1, 0:1],
                        start=True, stop=True)
                nc.vector.tensor_scalar_mul(gmlpT[:], psc3[:], 1.0)
                scr2 = rows4.tile([1, NQ], F32, name='scr2')
                nc.vector.tensor_mul(scr2[:], mu2[:], mu2[:])
                nc.vector.tensor_sub(ex22[:], ex22[:], scr2[:])
                nc.scalar.activation(scr2[:], ex22[:], AF.Sqrt,
                                     bias=eps_t[:])
                nc.vector.reciprocal_approx_fast(ex22[:], scr2[:])
                a2 = rows4.tile([1, NQ], BF16, name='a2')
                nc.vector.tensor_copy(a2[:], ex22[:])
                b2r = rows4.tile([1, NQ], BF16, name='b2r')
                nc.vector.scalar_tensor_tensor(b2r[:], mu2[:], -1.0,
                                               ex22[:], ALU.mult, ALU.mult)
                S2_row = rows4.tile([1, D], BF16, name='S2_row')
                nc.vector.tensor_scalar_add(S2_row[:],
                                            mod_row[0:1, 8 * NQ:10 * NQ],
                                            1.0)

                for k in range(KT):
                    pa = psab2.tile([P, NQ], F32, tag="pA2", name='pA2')
                    pb = psab2.tile([P, NQ], F32, tag="pB2", name='pB2')
                    nc.tensor.matmul(
                        pa[:], lhsT=S2_row[0:1, k * P:(k + 1) * P],
                        rhs=a2[:], start=True, stop=True)
                    nc.tensor.matmul(
                        pb[:], lhsT=S2_row[0:1, k * P:(k + 1) * P],
                        rhs=b2r[:], start=True, stop=False)
                    nc.tensor.matmul(
                        pb[:],
                        lhsT=mod_row[0:1, 6 * NQ + k * P:6 * NQ + (k + 1) * P],
                        rhs=ones_rowq[:], start=False, stop=True)
                    nc.vector.tensor_mul(h2t[k][:], x1t[k][:], pa[:])
                    nc.vector.tensor_add(h2t[k][:], h2t[k][:], pb[:])

        op_cm.__exit__(None, None, None)
        x_cm.__exit__(None, None, None)

        # ---------------- phase 5: W1 + gelu ----------------
        with ExitStack() as ph:
            ps1 = ph.enter_context(
                tc.tile_pool(name="ps1", bufs=4, space="PSUM"))

            for grp in range(GRP):
                if grp + 2 < GRP:
                    issue_w1_dma(grp + 2)
                th = w1tiles[grp]
                th3 = th.rearrange("p (k c) -> p k c", k=KT)
                for dot in range(4):
                    m = 4 * grp + dot
                    msl = slice(dot * P, (dot + 1) * P)
                    p = ps1.tile([P, NQ], F32, tag="m1", name='m1')
                    for k in range(KT):
                        nc.tensor.matmul(p[:], lhsT=th3[:, k, msl],
                                         rhs=h2t[k][:],
                                         start=(k == 0), stop=(k == KT - 1))
                    nc.scalar.activation(gTt[m][:], p[:],
                                         AF.Gelu_apprx_tanh,
                                         bias=b1T[:, m:m + 1])

        h2_cm.__exit__(None, None, None)

        # ---------------- phase 6: W2 (streamed) + output ----------------
        with ExitStack() as ph:
            opool = ph.enter_context(tc.tile_pool(name="p6o", bufs=3))
            ps2 = ph.enter_context(
                tc.tile_pool(name="ps2", bufs=1, space="PSUM"))

            for half in range(2):
                pacc = [ps2.tile([P, NQ], F32, tag=f"acc{d}",
                                 name=f"acc{d}") for d in range(4)]
                for mj in range(MT // 4):
                    base = (half * MT + 4 * mj) * NQ
                    w2c = w2p.tile([P, 4 * NQ], BF16, tag="w2", name='w2')
                    chained_dma(w2c[:], W2r[:, base:base + 4 * NQ],
                                e=(mj % 2))
                    w23 = w2c.rearrange("p (u c) -> p u c", u=4)
                    for u in range(4):
                        mk = 4 * mj + u
                        for d in range(4):
                            nc.tensor.matmul(
                                pacc[d][:],
                                lhsT=w23[:, u, d * P:(d + 1) * P],
                                rhs=gTt[mk][:],
                                start=(mk == 0), stop=(mk == MT - 1))
                for d in range(4):
                    tau = half * 4 + d
                    tmp = opool.tile([P, NQ], F32, tag="m2s", name='m2s')
                    nc.vector.tensor_scalar(tmp[:], pacc[d][:],
                                            b2T[:, tau:tau + 1],
                                            gmlpT[:, tau:tau + 1],
                                            ALU.add, ALU.mult)
                    yt = opool.tile([P, NQ], F32, tag="yout", name='yout')
                    nc.vector.tensor_add(yt[:], x1t[tau][:], tmp[:])
                    nc.sync.dma_start(yT[tau * P:(tau + 1) * P, :], yt[:])

        x1_cm.__exit__(None, None, None)
        g_cm.__exit__(None, None, None)

    nc.compile()
    return nc


_NC = None


def _get_nc():
    global _NC
    if _NC is None:
        _NC = build()
    return _NC


def _prep_inputs(x, c, Wq, bq, Wkv, bkv, Wo, bo, W1, b1, W2, b2, Wada, bada):
    import ml_dtypes
    f = np.float32
    bf = ml_dtypes.bfloat16
    f8 = ml_dtypes.float8_e4m3
    # (bf used for xbT below)

    def q8(v):
        return np.clip(np.asarray(v, f), -240.0, 240.0).astype(f8)

    def drp(W):
        # [K, M] -> [128, J*2*M] partition-major DoubleRow layout:
        # col j*(2M) + t*M + m  <->  W[(2j+t)*128 + p, m]
        K, M = W.shape
        J = K // (2 * P)
        return np.ascontiguousarray(
            W.reshape(J, 2, P, M).transpose(2, 0, 1, 3).reshape(P, J * 2 * M))

    def pgrp(W, ncol):
        # bf16 [K, M] -> [128, (g, k, ncol)]: col g*(K//128*ncol) + k*ncol+cc
        K, M = W.shape
        KTl = K // P
        G = M // ncol
        return np.ascontiguousarray(
            W.reshape(KTl, P, G, ncol).transpose(1, 2, 0, 3)
            .reshape(P, K * M // P))

    Wq8 = drp(q8(np.asarray(Wq, f) * 16.0))
    Wkv8 = drp(q8(np.asarray(Wkv, f) * 16.0))

    col = lambda v, n: np.ascontiguousarray(np.asarray(v, f).reshape(n, P).T)

    bo_eff = np.asarray(bkv, f)[D:] @ np.asarray(Wo, f) + np.asarray(bo, f)

    # W2 [MLPD, D] -> [128, (half, mk, 512)]
    W2f = np.asarray(W2, f).astype(bf)
    W2r = np.ascontiguousarray(
        W2f.reshape(MT, P, 2, NQ).transpose(1, 2, 0, 3).reshape(P, -1))

    shared = {
        "Wqd": Wq8, "Wkvd": Wkv8,
        "W1r": pgrp(np.asarray(W1, f).astype(bf), NQ),
        "W2r": W2r,
        "Wo": np.asarray(Wo, f).astype(bf),
        "Wada": np.asarray(Wada, f).astype(bf),
        "bada_r": np.asarray(bada, f).reshape(1, -1).astype(bf),
        "bq16_c": col(np.asarray(bq, f) * 16.0, KT),
        "bo16_c": col(bo_eff * 16.0, KT),
        "b1_c": col(b1, MT),
        "b2_c": col(b2, KT),
    }
    in_maps = []
    for core in range(NCORES):
        b, half = core // 2, core % 2
        xb = np.asarray(x[b], np.float32)
        perm_x = np.concatenate(
            [xb[half * NQ:(half + 1) * NQ],
             xb[(1 - half) * NQ:(2 - half) * NQ]], axis=0)
        m = dict(shared)
        m["xbT"] = np.ascontiguousarray(perm_x.T).astype(bf)
        m["xoT"] = np.ascontiguousarray(perm_x[0:NQ].T)
        m["crow"] = np.asarray(c[b:b + 1], np.float32)
        in_maps.append(m)
    return in_maps


def _run(inputs, trace=False):
    nc = _get_nc()
    in_maps = _prep_inputs(**inputs)
    res = run_bass_kernel_spmd(nc, in_maps, core_ids=list(range(NCORES)),
                               trace=trace)
    B = 4
    y = np.empty((B, N, D), np.float32)
    for core in range(NCORES):
        b, half = core // 2, core % 2
        y[b, half * NQ:(half + 1) * NQ, :] = res.results[core]["yT"].T
    return y, res


def kernel(**inputs):
    y, _ = _run(inputs, trace=False)
    return y
